# revision 24
# baseline (speedup 1.0000x reference)
"""Multi-head dot-product attention with prefix KV, on 8 trn2 NeuronCores.

Sharding: batch (2) x head-groups (4 groups of 4 heads) = 8 cores.
Each core computes q/k/v projections for its 4 heads, flash-style
attention (scores kept transposed: [kv, L] so no on-device transposes
are needed), and a partial out-projection [E, L]; the host sums the 4
head-group partials per batch and transposes back.

v2 layout/schedule notes:
  - all matmul operands are bf16 (PSUM accumulation stays fp32): halves
    LDWEIGHTS time and input DMA, and enables the fast DVE modes.
  - kv axis padded to 2176 = 17*128: chunk 0 = [prefix(64) | dead(64)],
    chunks 1..16 = kv positions.  Dead columns are killed with a
    per-partition -1e10 bias on the chunk-0 exp.
  - exact causal q-trim: per (group, chunk) the score/exp/ctx work only
    covers the valid q-range [qlo, 512); the elementwise mask multiply
    shrinks to the (deduped) mixed window (a [128,128] triangle for the
    causal mask).
  - softmax runs without max subtraction (scores are O(1)); denominator
    comes free as a ones-column in the v weights (M=65 ctx matmul);
    per-group denominators are gathered to partitions {0,32,64,96},
    reciprocal'd with one fast-approx DVE op, and broadcast across
    partitions with K=1 outer-product matmuls.
  - q/k projections use N=512 matmuls (half the instruction count);
    projection and out-projection matmuls are interleaved between the
    score and ctx matmuls of every attention group as PE filler, so the
    tensor engine never waits on the softmax (ACT) pipeline.
"""

import numpy as np
import ml_dtypes

BF16 = ml_dtypes.bfloat16

B, LQ, LKV, E, H, D, P = 2, 2048, 2048, 1024, 16, 64, 64
NCORES = 8
HGROUPS = 4          # head groups (cores per batch)
HPC = H // HGROUPS   # heads per core = 4
KVPAD = 128 + LKV    # 2176
NCH = KVPAD // 128   # 17 chunks
NG = LQ // 512       # 4 L-groups of 512
NEG = -1.0e10

_CACHE = {}


def _build_module(plan, debug_taps=False):
    """Build the single-core Bass module (same program for all 8 cores)."""
    import concourse.bass as bass
    import concourse.tile as tile
    import concourse.mybir as mybir
    from concourse import bacc
    from contextlib import ExitStack

    f32 = mybir.dt.float32
    bf16 = mybir.dt.bfloat16
    Exp = mybir.ActivationFunctionType.Exp

    chunks = plan["chunks"]        # g -> [c...] ascending, 0 first
    qlo = plan["qlo"]              # (g,c) -> valid-q start col (0..511)
    win = plan["win"]              # (g,c) -> (mlo, mhi, tile_idx) or absent
    ntiles = plan["ntiles"]

    nc = bacc.Bacc("TRN2", target_bir_lowering=False, debug=False,
                   enable_asserts=False, num_devices=NCORES)

    xqT_d = nc.dram_tensor("xqT", [E, LQ], bf16, kind="ExternalInput").ap()
    xkvT_d = nc.dram_tensor("xkvT", [E, LKV], bf16, kind="ExternalInput").ap()
    wq_d = nc.dram_tensor("wq", [E, HPC * D], bf16, kind="ExternalInput").ap()
    wk_d = nc.dram_tensor("wk", [E, HPC * D], bf16, kind="ExternalInput").ap()
    wv_d = nc.dram_tensor("wv", [E, HPC * D], bf16, kind="ExternalInput").ap()
    wo_d = nc.dram_tensor("wo", [HPC * D, E], bf16, kind="ExternalInput").ap()
    kprefT_d = nc.dram_tensor("kprefT", [2, 128, 128], bf16, kind="ExternalInput").ap()
    vpref_d = nc.dram_tensor("vpref", [128, HPC, D], bf16, kind="ExternalInput").ap()
    if ntiles:
        maskblk_d = nc.dram_tensor("maskblk", [ntiles, 128, 128], bf16,
                                   kind="ExternalInput").ap()
    outT_d = nc.dram_tensor("outT", [E, LQ], bf16, kind="ExternalOutput").ap()

    with tile.TileContext(nc) as tc, ExitStack() as stk:
        pers = stk.enter_context(tc.tile_pool(name="pers", bufs=1))

        def ptile(shape, name, dt=bf16):
            return pers.tile(shape, dt, tag=name, name=name)

        wq_sb = ptile([128, 8, 256], "wq_sb")
        wk_sb = ptile([128, 8, 256], "wk_sb")
        wv_sb = ptile([128, 8, 256], "wv_sb")
        wo_sb = ptile([128, 2, 1024], "wo_sb")
        # per-L-group tensors (512 wide); K prefix is its own [128,128] tile
        QTS = [[ptile([128, 512], f"QT{i}g{g}") for g in range(NG)] for i in range(2)]
        KTS = [[ptile([128, 512], f"KT{i}g{g}") for g in range(NG)] for i in range(2)]
        KPR = [ptile([128, 128], f"KP{i}") for i in range(2)]
        VTS = [ptile([128, HPC, 65], f"VT{c}") for c in range(NCH)]
        CTXT = [[ptile([128, 512], f"CTXT{i}g{g}") for g in range(NG)]
                for i in range(2)]
        cb0 = ptile([128, 1], "cb0", f32)
        ones_col = ptile([128, 64], "ones_col")
        ones_f32 = ptile([128, 64], "ones_f32", f32)
        mts = [ptile([128, 128], f"mt{i}") for i in range(ntiles)]

        def kslice(hc, c):
            if c == 0:
                return KPR[hc]
            g, off = (c - 1) // 4, 128 * ((c - 1) % 4)
            return KTS[hc][g][:, off:off + 128]

        xio = stk.enter_context(tc.tile_pool(name="xio", bufs=2))

        def proj_load(g):
            l0 = 512 * g
            xq_t = xio.tile([128, 8, 512], bf16, tag="xq", bufs=2, name="xq_t")
            xkv_t = xio.tile([128, 8, 512], bf16, tag="xkv", bufs=2, name="xkv_t")
            nc.sync.dma_start(
                out=xq_t,
                in_=xqT_d.rearrange("(ec p) l -> p ec l", p=128)[:, :, l0:l0 + 512])
            nc.sync.dma_start(
                out=xkv_t,
                in_=xkvT_d.rearrange("(ec p) l -> p ec l", p=128)[:, :, l0:l0 + 512])
            return xq_t, xkv_t

        # startup DMAs ordered by first consumption: wq+x(0) gate proj(0)
        nc.sync.dma_start(out=wq_sb, in_=wq_d.rearrange("(ec p) m -> p ec m", p=128))
        ld0 = proj_load(0)
        nc.sync.dma_start(out=wk_sb, in_=wk_d.rearrange("(ec p) m -> p ec m", p=128))
        nc.sync.dma_start(out=wv_sb, in_=wv_d.rearrange("(ec p) m -> p ec m", p=128))
        nc.sync.dma_start(out=wo_sb, in_=wo_d.rearrange("(hc p) e -> p hc e", p=128))

        nc.vector.memset(cb0[0:64, :], 0.0)
        nc.vector.memset(cb0[64:128, :], NEG)
        nc.vector.memset(ones_col, 1.0)
        nc.vector.memset(ones_f32, 1.0)
        for c in range(NCH):
            nc.vector.memset(VTS[c][:, :, 64:65], 1.0)
        for i in range(ntiles):
            nc.sync.dma_start(out=mts[i], in_=maskblk_d[i])
        for hc in range(2):
            nc.sync.dma_start(out=KPR[hc], in_=kprefT_d[hc])
        nc.sync.dma_start(out=VTS[0][:, :, 0:D], in_=vpref_d)

        attps = stk.enter_context(tc.tile_pool(name="att_ps", bufs=1, space="PSUM"))
        attsb = stk.enter_context(tc.tile_pool(name="att_sb", bufs=1))
        pjps = stk.enter_context(tc.tile_pool(name="pj_ps", bufs=1, space="PSUM"))

        def proj_units(g, loaded):
            """q/k/v projection for L-group g as a sequence of 4-matmul
            quanta (yield between quanta so attention can interleave)."""
            xq_t, xkv_t = loaded
            # each unit is atomic: its 8-matmul PSUM accumulation group must
            # close before anything else can allocate from the pj ring
            for t in range(2):
                for w_sb, x_t, dst in ((wq_sb, xq_t, QTS), (wk_sb, xkv_t, KTS)):
                    ps = pjps.tile([128, 512], f32, tag="pj", bufs=2, name="ps_p")
                    for ec in range(8):
                        nc.tensor.matmul(
                            ps, lhsT=w_sb[:, ec, 128 * t:128 * t + 128],
                            rhs=x_t[:, ec, :], start=(ec == 0), stop=(ec == 7))
                    nc.vector.tensor_copy(out=dst[t][g], in_=ps)
                    yield
            for sub in range(4):
                ps = pjps.tile([128, 512], f32, tag="pj", bufs=2, name="ps_v")
                for ec in range(8):
                    nc.tensor.matmul(
                        ps[:, 0:256], lhsT=xkv_t[:, ec, 128 * sub:128 * sub + 128],
                        rhs=wv_sb[:, ec, :], start=(ec == 0), stop=(ec == 7))
                nc.vector.tensor_copy(
                    out=VTS[4 * g + sub + 1][:, :, 0:D],
                    in_=ps[:, 0:256].rearrange("p (h d) -> p h d", h=HPC))
                yield

        op_n = [0]

        def outproj_units(g):
            """out-projection for L-group g: 8 units of (2 matmuls + copy)."""
            gl = 512 * g
            for et in range(8):
                ops = pjps.tile([128, 512], f32, tag="pj", bufs=2, name="ops")
                for hc in range(2):
                    nc.tensor.matmul(
                        ops, lhsT=wo_sb[:, hc, 128 * et:128 * et + 128],
                        rhs=CTXT[hc][g], start=(hc == 0), stop=(hc == 1))
                ot = attsb.tile([128, 512], bf16, tag="ostage", bufs=3, name="ot")
                # PSUM->SBUF copy on DVE (ACT is saturated by the softmax
                # exps mid-run); the last group's units alternate onto ACT,
                # which is idle during the tail
                if g == NG - 1 and op_n[0] % 2 == 0:
                    nc.scalar.copy(ot, ops)
                else:
                    nc.vector.tensor_copy(out=ot, in_=ops)
                op_n[0] += 1
                nc.sync.dma_start(
                    out=outT_d[128 * et:128 * et + 128, gl:gl + 512], in_=ot)
                yield

        class FillerQ:
            """Two queues of generators (proj must flush at group
            boundaries, outproj carries over); step() advances one quantum,
            preferring proj."""
            def __init__(self):
                self.proj = []
                self.op = []

            @staticmethod
            def _step_one(q):
                while q:
                    try:
                        next(q[0])
                        return True
                    except StopIteration:
                        q.pop(0)
                return False

            def step(self, n=1):
                for _ in range(n):
                    if not self._step_one(self.proj):
                        self._step_one(self.op)

            def flush_proj(self):
                while self._step_one(self.proj):
                    pass

            def drain_all(self):
                self.flush_proj()
                while self._step_one(self.op):
                    pass

        fill = FillerQ()

        def denb_head(g, h, cx):
            """Per-head denominator finish: partition-broadcast the raw
            denom row via a K=1 matmul, fast reciprocal (at partition
            offset 0 — the custom DVE op needs that), CTXT scaling."""
            hc, par = h // 2, h % 2
            bc_ps = pjps.tile([128, 512], f32, tag="pj", bufs=2, name="bc_ps")
            nc.tensor.matmul(bc_ps[0:64, :], lhsT=ones_f32[64:65, :],
                             rhs=cx[64:65, :], start=True, stop=True,
                             tile_position=(64, 0))
            rcs = attsb.tile([64, 512], f32, tag="rcs", bufs=2, name="rcs")
            nc.vector.reciprocal_approx_fast(out=rcs, in_=bc_ps[0:64, :])
            if par == 0:
                nc.vector.tensor_mul(CTXT[hc][g][0:64, :], cx[0:64, :], rcs)
            else:
                st = attsb.tile([64, 512], bf16, tag="stage", bufs=2, name="st")
                nc.vector.tensor_mul(st, cx[0:64, :], rcs)
                nc.sync.dma_start(out=CTXT[hc][g][64:128, :], in_=st)

        def attn_group(g, pending_cb=None):
            """Score/softmax/ctx for group g.  Returns a closure that
            finishes the softmax denominators (reciprocal + broadcast +
            CTXT scaling); the caller runs it early in the NEXT group so
            the in-order PE queue never stalls on the recip sem chain."""
            cs = chunks[g]
            batches = [[cs[0]]] + [cs[1 + i:3 + i] for i in range(0, len(cs) - 1, 2)]
            ctxs = {}
            if g < NG - 1:
                denoms4 = attsb.tile([97, 512], f32, tag="den4", bufs=2,
                                     name="denoms4")
                nc.vector.memset(denoms4, 1.0)
            nbat = len(batches)
            for h in range(HPC):
                hc, prow = h // 2, 64 * (h % 2)
                ctx_ps = attps.tile([65, 512], f32, tag="ctx", bufs=2,
                                    name=f"ctx{h}")
                for bi, batch in enumerate(batches):
                    sc = attps.tile([128, 1024], f32, tag="sc", bufs=2,
                                    name=f"sc{h}")
                    for j, c in enumerate(batch):
                        q0 = qlo[(g, c)]
                        nc.tensor.matmul(
                            sc[:, 512 * j + q0:512 * j + 512],
                            lhsT=kslice(hc, c)[prow:prow + 64, :],
                            rhs=QTS[hc][g][prow:prow + 64, q0:512],
                            start=True, stop=True)
                    pr = attsb.tile([128, 1024], bf16, tag="pr", bufs=3,
                                    name=f"pr{h}")
                    e0 = qlo[(g, batch[0])]
                    we = 512 * len(batch)
                    if batch[0] == 0:
                        nc.scalar.activation(pr[:, e0:we], sc[:, e0:we],
                                             Exp, bias=cb0[:, 0:1])
                    else:
                        nc.scalar.activation(pr[:, e0:we], sc[:, e0:we], Exp)
                    fill.step(1)
                    for j, c in enumerate(batch):
                        for mlo, mhi, ti in win.get((g, c), ()):
                            nc.gpsimd.tensor_mul(
                                pr[:, 512 * j + mlo:512 * j + mhi],
                                pr[:, 512 * j + mlo:512 * j + mhi],
                                mts[ti][:, 0:mhi - mlo])
                    for j, c in enumerate(batch):
                        q0 = qlo[(g, c)]
                        nc.tensor.matmul(
                            ctx_ps[:, q0:512],
                            lhsT=VTS[c][:, h, :],
                            rhs=pr[:, 512 * j + q0:512 * j + 512],
                            start=(bi == 0 and j == 0),
                            stop=(bi == nbat - 1 and j == len(batch) - 1))
                    fill.step(1)
                    if pending_cb is not None and h == 0 and bi == 1:
                        pending_cb()
                        pending_cb = None
                # copy ctx+denom to SBUF to release the PSUM bank
                ctxs[h] = attsb.tile([65, 512], f32, tag="ctxs", bufs=8,
                                     name=f"ctxs{h}")
                nc.vector.tensor_copy(out=ctxs[h], in_=ctx_ps)
                if g == NG - 1:
                    # last group: per-head denominator finish so the tail
                    # isn't serialized behind a batched recip chain
                    denb_head(g, h, ctxs[h])
                else:
                    # gather the denom row into denoms4 at partition 32h
                    # for the batched (deferred) per-group reciprocal
                    nc.sync.dma_start(out=denoms4[32 * h:32 * h + 1, :],
                                      in_=ctxs[h][64:65, :])
            if pending_cb is not None:
                pending_cb()

            if g == NG - 1:
                return lambda: None

            def finish():
                rc4 = attsb.tile([97, 512], f32, tag="rc4", bufs=2, name="rc4")
                nc.vector.reciprocal_approx_fast(out=rc4, in_=denoms4)
                rc4b = attsb.tile([97, 512], bf16, tag="rc4b", bufs=2, name="rc4b")
                nc.vector.tensor_copy(out=rc4b, in_=rc4)
                for h in range(HPC):
                    hc, par = h // 2, h % 2
                    bc_ps = pjps.tile([128, 512], f32, tag="pj", bufs=2,
                                      name="bc_ps")
                    nc.tensor.matmul(bc_ps[0:64, :],
                                     lhsT=ones_col[32 * h:32 * h + 1, :],
                                     rhs=rc4b[32 * h:32 * h + 1, :],
                                     start=True, stop=True,
                                     tile_position=(32 * h, 0))
                    if par == 0:
                        nc.vector.tensor_mul(CTXT[hc][g][0:64, :],
                                             ctxs[h][0:64, :], bc_ps[0:64, :])
                    else:
                        st = attsb.tile([64, 512], bf16, tag="stage", bufs=2,
                                        name="st")
                        nc.vector.tensor_mul(st, ctxs[h][0:64, :], bc_ps[0:64, :])
                        nc.sync.dma_start(out=CTXT[hc][g][64:128, :], in_=st)
            return finish

        # schedule: proj(0) upfront; during attn(g) the filler queue holds
        # proj(g+1); out-projections fill the (otherwise PE-starved) late
        # groups: op(0) during attn(2), op(1)/op(2) during attn(3); proj
        # leftovers flush at the group boundary (they gate attn(g+1)).
        for _ in proj_units(0, ld0):
            pass
        pending = None
        for g in range(NG):
            if g + 1 < NG:
                ld = proj_load(g + 1)
                fill.proj.append(proj_units(g + 1, ld))
            if g == NG - 2:
                fill.op.append(outproj_units(0))
            if g == NG - 1:
                fill.op.append(outproj_units(1))
                # outproj(2) may only be emitted after denb(2) has written
                # CTXT[*][2]; chain it onto the deferred finish callback
                prev = pending

                def pending(prev=prev):
                    prev()
                    fill.op.append(outproj_units(NG - 2))
            pending = attn_group(g, pending)
            if g + 1 < NG:
                fill.flush_proj()
        pending()
        fill.drain_all()
        for _ in outproj_units(NG - 1):
            pass

    nc.compile()
    return nc


def _make_plan(mask):
    """Block plan from the actual mask (union over batches -> one SPMD plan).

    For each (q-group g, kv-chunk c) computes:
      - inclusion (any valid element),
      - qlo: first q column (within the group's 512) with any valid kv,
      - the mixed window [mlo, mhi) of q columns that need an elementwise
        mask multiply, with deduped [128, mhi-mlo<=128...] tiles.
    """
    m = np.asarray(mask[:, 0])                       # [B, LQ, LKV] bool
    chunks, qlo, wins = [], {}, {}
    uniq, order = {}, []                             # content-hash -> idx
    for g in range(NG):
        cl = [0]
        qlo[(g, 0)] = 0
        for c in range(1, NCH):
            blk = m[:, 512 * g:512 * g + 512, 128 * (c - 1):128 * c]  # [B,512,128]
            anyk = blk.any(axis=2)                   # [B, 512]
            if not anyk.any():
                continue
            cl.append(c)
            valid_cols = anyk.any(axis=0)            # union over batches
            q0 = int(np.argmax(valid_cols))
            qlo[(g, c)] = q0
            allk = blk.all(axis=2).all(axis=0)       # [512] all-valid cols
            mixed = valid_cols & ~allk
            if mixed.any():
                lo = int(np.argmax(mixed))
                hi = 512 - int(np.argmax(mixed[::-1]))
                wl = []
                # split into <=128-wide windows (mask tiles are [128,128])
                for mlo in range(lo, hi, 128):
                    mhi = min(mlo + 128, hi)
                    # per-batch tile content; dedup on the union key so all
                    # cores run the same program with per-core data
                    key = (mhi - mlo, blk[:, mlo:mhi, :].tobytes())
                    if key not in uniq:
                        uniq[key] = len(order)
                        order.append((g, c, mlo, mhi))
                    wl.append((mlo, mhi, uniq[key]))
                wins[(g, c)] = wl
        chunks.append(cl)
    return {"chunks": chunks, "qlo": qlo, "win": wins, "ntiles": len(order),
            "order": order}


def _prep_core_inputs(inputs, plan):
    """Per-core input dicts (8 cores: batch-major, then head-group)."""
    inputs_q = np.asarray(inputs["inputs_q"], dtype=np.float32)
    inputs_kv = np.asarray(inputs["inputs_kv"], dtype=np.float32)
    key_prefix = np.asarray(inputs["key_prefix"], dtype=np.float32)
    value_prefix = np.asarray(inputs["value_prefix"], dtype=np.float32)
    mask = np.asarray(inputs["mask"])
    Wq = np.asarray(inputs["Wq"], dtype=np.float32)
    Wk = np.asarray(inputs["Wk"], dtype=np.float32)
    Wv = np.asarray(inputs["Wv"], dtype=np.float32)
    Wo = np.asarray(inputs["Wo"], dtype=np.float32)

    xT = [np.ascontiguousarray(inputs_q[b].T.astype(BF16)) for b in range(B)]
    xkT = [np.ascontiguousarray(inputs_kv[b].T.astype(BF16)) for b in range(B)]

    maskblks = []
    ntiles = plan["ntiles"]
    for b in range(B):
        mb = np.zeros((max(ntiles, 1), 128, 128), np.float32)
        for i, (g, c, mlo, mhi) in enumerate(plan["order"]):
            mb[i, :, 0:mhi - mlo] = mask[
                b, 0, 512 * g + mlo:512 * g + mhi,
                128 * (c - 1):128 * c].T.astype(np.float32)
        maskblks.append(mb.astype(BF16))

    in_maps = []
    for core in range(NCORES):
        b, hg = core // HGROUPS, core % HGROUPS
        hs = slice(HPC * hg, HPC * (hg + 1))
        kpT = key_prefix[b, :, hs, :]                 # [P, 4, D]
        kpT = kpT.transpose(1, 2, 0).reshape(2, 128, P)  # [hc, (2 heads x D), P]
        kpT = np.concatenate(
            [kpT, np.zeros((2, 128, 128 - P), np.float32)], axis=2)
        im = {
            "xqT": xT[b],
            "xkvT": xkT[b],
            "wq": np.ascontiguousarray(
                (Wq[:, hs, :] / np.sqrt(D)).reshape(E, HPC * D).astype(BF16)),
            "wk": np.ascontiguousarray(Wk[:, hs, :].reshape(E, HPC * D).astype(BF16)),
            "wv": np.ascontiguousarray(Wv[:, hs, :].reshape(E, HPC * D).astype(BF16)),
            "wo": np.ascontiguousarray(Wo[hs].reshape(HPC * D, E).astype(BF16)),
            "kprefT": np.ascontiguousarray(kpT.astype(BF16)),
            "vpref": np.ascontiguousarray(np.concatenate(
                [value_prefix[b, :, hs, :],
                 np.zeros((128 - P, HPC, D), np.float32)], axis=0).astype(BF16)),
        }
        if ntiles:
            im["maskblk"] = maskblks[b]
        in_maps.append(im)
    return in_maps


def kernel(**inputs) -> np.ndarray:
    from concourse import bass_utils

    plan = _make_plan(inputs["mask"])
    key = (tuple(tuple(c) for c in plan["chunks"]),
           tuple(sorted(plan["qlo"].items())),
           tuple(sorted((k, tuple(v)) for k, v in plan["win"].items())),
           plan["ntiles"])
    if key not in _CACHE:
        _CACHE[key] = _build_module(plan)
    nc = _CACHE[key]

    in_maps = _prep_core_inputs(inputs, plan)
    res = bass_utils.run_bass_kernel_spmd(nc, in_maps, core_ids=list(range(NCORES)))

    out = np.zeros((B, LQ, E), np.float32)
    for core in range(NCORES):
        b = core // HGROUPS
        out[b] += res.results[core]["outT"].T.astype(np.float32)
    return out


# revision 25
# speedup vs baseline: 1.1278x; 1.1278x over previous
"""Multi-head dot-product attention with prefix KV, on 8 trn2 NeuronCores.

Sharding: batch (2) x head-groups (4 groups of 4 heads) = 8 cores.
Each core computes q/k/v projections for its 4 heads, flash-style
attention (scores kept transposed: [kv, L] so no on-device transposes
are needed), and a partial out-projection [E, L]; the host sums the 4
head-group partials per batch and transposes back.

v2 layout/schedule notes:
  - all matmul operands are bf16 (PSUM accumulation stays fp32): halves
    LDWEIGHTS time and input DMA, and enables the fast DVE modes.
  - kv axis padded to 2176 = 17*128: chunk 0 = [prefix(64) | dead(64)],
    chunks 1..16 = kv positions.  Dead columns are killed with a
    per-partition -1e10 bias on the chunk-0 exp.
  - exact causal q-trim: per (group, chunk) the score/exp/ctx work only
    covers the valid q-range [qlo, 512); the elementwise mask multiply
    shrinks to the (deduped) mixed window (a [128,128] triangle for the
    causal mask).
  - softmax runs without max subtraction (scores are O(1)); denominator
    comes free as a ones-column in the v weights (M=65 ctx matmul);
    per-group denominators are gathered to partitions {0,32,64,96},
    reciprocal'd with one fast-approx DVE op, and broadcast across
    partitions with K=1 outer-product matmuls.
  - q/k projections use N=512 matmuls (half the instruction count);
    projection and out-projection matmuls are interleaved between the
    score and ctx matmuls of every attention group as PE filler, so the
    tensor engine never waits on the softmax (ACT) pipeline.
"""

import numpy as np
import ml_dtypes

BF16 = ml_dtypes.bfloat16

B, LQ, LKV, E, H, D, P = 2, 2048, 2048, 1024, 16, 64, 64
NCORES = 8
HGROUPS = 4          # head groups (cores per batch)
HPC = H // HGROUPS   # heads per core = 4
KVPAD = 128 + LKV    # 2176
NCH = KVPAD // 128   # 17 chunks
NG = LQ // 512       # 4 L-groups of 512
NEG = -1.0e10

_CACHE = {}


def _build_module(plan, debug_taps=False):
    """Build the single-core Bass module (same program for all 8 cores)."""
    import concourse.bass as bass
    import concourse.tile as tile
    import concourse.mybir as mybir
    from concourse import bacc
    from contextlib import ExitStack

    f32 = mybir.dt.float32
    bf16 = mybir.dt.bfloat16
    Exp = mybir.ActivationFunctionType.Exp

    chunks = plan["chunks"]        # g -> [c...] ascending, 0 first
    qlo = plan["qlo"]              # (g,c) -> valid-q start col (0..511)
    win = plan["win"]              # (g,c) -> (mlo, mhi, tile_idx) or absent
    ntiles = plan["ntiles"]

    nc = bacc.Bacc("TRN2", target_bir_lowering=False, debug=False,
                   enable_asserts=False, num_devices=NCORES)

    xqT_d = nc.dram_tensor("xqT", [E, LQ], bf16, kind="ExternalInput").ap()
    xkvT_d = nc.dram_tensor("xkvT", [E, LKV], bf16, kind="ExternalInput").ap()
    wq_d = nc.dram_tensor("wq", [E, HPC * D], bf16, kind="ExternalInput").ap()
    wk_d = nc.dram_tensor("wk", [E, HPC * D], bf16, kind="ExternalInput").ap()
    wv_d = nc.dram_tensor("wv", [E, HPC * D], bf16, kind="ExternalInput").ap()
    wo_d = nc.dram_tensor("wo", [HPC * D, E], bf16, kind="ExternalInput").ap()
    kprefT_d = nc.dram_tensor("kprefT", [2, 128, 128], bf16, kind="ExternalInput").ap()
    vpref_d = nc.dram_tensor("vpref", [128, HPC, D], bf16, kind="ExternalInput").ap()
    if ntiles:
        maskblk_d = nc.dram_tensor("maskblk", [ntiles, 128, 128], bf16,
                                   kind="ExternalInput").ap()
    outT_d = nc.dram_tensor("outT", [E, LQ], bf16, kind="ExternalOutput").ap()

    with tile.TileContext(nc) as tc, ExitStack() as stk:
        pers = stk.enter_context(tc.tile_pool(name="pers", bufs=1))

        def ptile(shape, name, dt=bf16):
            return pers.tile(shape, dt, tag=name, name=name)

        wq_sb = ptile([128, 8, 256], "wq_sb")
        wk_sb = ptile([128, 8, 256], "wk_sb")
        wv_sb = ptile([128, 8, 256], "wv_sb")
        wo_sb = ptile([128, 2, 1024], "wo_sb")
        # per-L-group tensors (512 wide); K prefix is its own [128,128] tile
        QTS = [[ptile([128, 512], f"QT{i}g{g}") for g in range(NG)] for i in range(2)]
        KTS = [[ptile([128, 512], f"KT{i}g{g}") for g in range(NG)] for i in range(2)]
        KPR = [ptile([128, 128], f"KP{i}") for i in range(2)]
        VTS = [ptile([128, HPC, 65], f"VT{c}") for c in range(NCH)]
        CTXT = [[ptile([128, 512], f"CTXT{i}g{g}") for g in range(NG)]
                for i in range(2)]
        cb0 = ptile([128, 1], "cb0", f32)
        ones_col = ptile([128, 64], "ones_col")
        ones_f32 = ptile([128, 64], "ones_f32", f32)
        mts = [ptile([128, 128], f"mt{i}") for i in range(ntiles)]

        def kslice(hc, c):
            if c == 0:
                return KPR[hc]
            g, off = (c - 1) // 4, 128 * ((c - 1) % 4)
            return KTS[hc][g][:, off:off + 128]

        xio = stk.enter_context(tc.tile_pool(name="xio", bufs=2))

        def proj_load(g):
            l0 = 512 * g
            xq_t = xio.tile([128, 8, 512], bf16, tag="xq", bufs=2, name="xq_t")
            xkv_t = xio.tile([128, 8, 512], bf16, tag="xkv", bufs=2, name="xkv_t")
            nc.sync.dma_start(
                out=xq_t,
                in_=xqT_d.rearrange("(ec p) l -> p ec l", p=128)[:, :, l0:l0 + 512])
            nc.sync.dma_start(
                out=xkv_t,
                in_=xkvT_d.rearrange("(ec p) l -> p ec l", p=128)[:, :, l0:l0 + 512])
            return xq_t, xkv_t

        # startup DMAs ordered by first consumption: wq+x(0) gate proj(0)
        nc.sync.dma_start(out=wq_sb, in_=wq_d.rearrange("(ec p) m -> p ec m", p=128))
        ld0 = proj_load(0)
        nc.sync.dma_start(out=wk_sb, in_=wk_d.rearrange("(ec p) m -> p ec m", p=128))
        nc.sync.dma_start(out=wv_sb, in_=wv_d.rearrange("(ec p) m -> p ec m", p=128))
        nc.sync.dma_start(out=wo_sb, in_=wo_d.rearrange("(hc p) e -> p hc e", p=128))

        nc.vector.memset(cb0[0:64, :], 0.0)
        nc.vector.memset(cb0[64:128, :], NEG)
        nc.vector.memset(ones_col, 1.0)
        nc.vector.memset(ones_f32, 1.0)
        for c in range(NCH):
            nc.vector.memset(VTS[c][:, :, 64:65], 1.0)
        for i in range(ntiles):
            nc.sync.dma_start(out=mts[i], in_=maskblk_d[i])
        for hc in range(2):
            nc.sync.dma_start(out=KPR[hc], in_=kprefT_d[hc])
        nc.sync.dma_start(out=VTS[0][:, :, 0:D], in_=vpref_d)

        attps = stk.enter_context(tc.tile_pool(name="att_ps", bufs=1, space="PSUM"))
        attsb = stk.enter_context(tc.tile_pool(name="att_sb", bufs=1))
        pjps = stk.enter_context(tc.tile_pool(name="pj_ps", bufs=1, space="PSUM"))

        def proj_units(g, loaded):
            """q/k/v projection for L-group g as a sequence of 4-matmul
            quanta (yield between quanta so attention can interleave)."""
            xq_t, xkv_t = loaded
            # each unit is atomic: its 8-matmul PSUM accumulation group must
            # close before anything else can allocate from the pj ring
            for t in range(2):
                for w_sb, x_t, dst in ((wq_sb, xq_t, QTS), (wk_sb, xkv_t, KTS)):
                    ps = pjps.tile([128, 512], f32, tag="pj", bufs=2, name="ps_p")
                    for ec in range(8):
                        nc.tensor.matmul(
                            ps, lhsT=w_sb[:, ec, 128 * t:128 * t + 128],
                            rhs=x_t[:, ec, :], start=(ec == 0), stop=(ec == 7))
                    nc.vector.tensor_copy(out=dst[t][g], in_=ps)
                    yield
            for sub in range(4):
                ps = pjps.tile([128, 512], f32, tag="pj", bufs=2, name="ps_v")
                for ec in range(8):
                    nc.tensor.matmul(
                        ps[:, 0:256], lhsT=xkv_t[:, ec, 128 * sub:128 * sub + 128],
                        rhs=wv_sb[:, ec, :], start=(ec == 0), stop=(ec == 7))
                nc.vector.tensor_copy(
                    out=VTS[4 * g + sub + 1][:, :, 0:D],
                    in_=ps[:, 0:256].rearrange("p (h d) -> p h d", h=HPC))
                yield

        op_n = [0]

        def outproj_units(g):
            """out-projection for L-group g: 8 units of (2 matmuls + copy)."""
            gl = 512 * g
            for et in range(8):
                ops = pjps.tile([128, 512], f32, tag="pj", bufs=2, name="ops")
                for hc in range(2):
                    nc.tensor.matmul(
                        ops, lhsT=wo_sb[:, hc, 128 * et:128 * et + 128],
                        rhs=CTXT[hc][g], start=(hc == 0), stop=(hc == 1))
                ot = attsb.tile([128, 512], bf16, tag="ostage", bufs=3, name="ot")
                # PSUM->SBUF copy on DVE (ACT is saturated by the softmax
                # exps mid-run); the last group's units alternate onto ACT,
                # which is idle during the tail
                if g == NG - 1 and op_n[0] % 2 == 0:
                    nc.scalar.copy(ot, ops)
                else:
                    nc.vector.tensor_copy(out=ot, in_=ops)
                op_n[0] += 1
                nc.sync.dma_start(
                    out=outT_d[128 * et:128 * et + 128, gl:gl + 512], in_=ot)
                yield

        class FillerQ:
            """Two queues of generators (proj must flush at group
            boundaries, outproj carries over); step() advances one quantum,
            preferring proj."""
            def __init__(self):
                self.proj = []
                self.op = []

            @staticmethod
            def _step_one(q):
                while q:
                    try:
                        next(q[0])
                        return True
                    except StopIteration:
                        q.pop(0)
                return False

            def step(self, n=1):
                for _ in range(n):
                    if not self._step_one(self.proj):
                        self._step_one(self.op)

            def flush_proj(self):
                while self._step_one(self.proj):
                    pass

            def drain_all(self):
                self.flush_proj()
                while self._step_one(self.op):
                    pass

        fill = FillerQ()

        def denb_head(g, h, cx):
            """Per-head denominator finish: partition-broadcast the raw
            denom row via a K=1 matmul, fast reciprocal (at partition
            offset 0 — the custom DVE op needs that), CTXT scaling."""
            hc, par = h // 2, h % 2
            bc_ps = pjps.tile([128, 512], f32, tag="pj", bufs=2, name="bc_ps")
            nc.tensor.matmul(bc_ps[0:64, :], lhsT=ones_f32[64:65, :],
                             rhs=cx[64:65, :], start=True, stop=True,
                             tile_position=(64, 0))
            rcs = attsb.tile([64, 512], f32, tag="rcs", bufs=2, name="rcs")
            nc.vector.reciprocal_approx_fast(out=rcs, in_=bc_ps[0:64, :])
            if par == 0:
                nc.vector.tensor_mul(CTXT[hc][g][0:64, :], cx[0:64, :], rcs)
            else:
                st = attsb.tile([64, 512], bf16, tag="stage", bufs=2, name="st")
                nc.vector.tensor_mul(st, cx[0:64, :], rcs)
                nc.sync.dma_start(out=CTXT[hc][g][64:128, :], in_=st)

        def attn_group(g, pending_cb=None):
            """Score/softmax/ctx for group g.  Returns a closure that
            finishes the softmax denominators (reciprocal + broadcast +
            CTXT scaling); the caller runs it early in the NEXT group so
            the in-order PE queue never stalls on the recip sem chain."""
            cs = chunks[g]
            batches = [[cs[0]]] + [cs[1 + i:3 + i] for i in range(0, len(cs) - 1, 2)]
            ctxs = {}
            if g < NG - 1:
                denoms4 = attsb.tile([97, 512], f32, tag="den4", bufs=2,
                                     name="denoms4")
                nc.vector.memset(denoms4, 1.0)
            nbat = len(batches)
            for h in range(HPC):
                hc, prow = h // 2, 64 * (h % 2)
                ctx_ps = attps.tile([65, 512], f32, tag="ctx", bufs=2,
                                    name=f"ctx{h}")
                for bi, batch in enumerate(batches):
                    sc = attps.tile([128, 1024], f32, tag="sc", bufs=2,
                                    name=f"sc{h}")
                    for j, c in enumerate(batch):
                        q0 = qlo[(g, c)]
                        nc.tensor.matmul(
                            sc[:, 512 * j + q0:512 * j + 512],
                            lhsT=kslice(hc, c)[prow:prow + 64, :],
                            rhs=QTS[hc][g][prow:prow + 64, q0:512],
                            start=True, stop=True)
                    pr = attsb.tile([128, 1024], bf16, tag="pr", bufs=3,
                                    name=f"pr{h}")
                    e0 = qlo[(g, batch[0])]
                    we = 512 * len(batch)
                    if batch[0] == 0:
                        nc.scalar.activation(pr[:, e0:we], sc[:, e0:we],
                                             Exp, bias=cb0[:, 0:1])
                    else:
                        nc.scalar.activation(pr[:, e0:we], sc[:, e0:we], Exp)
                    fill.step(1)
                    for j, c in enumerate(batch):
                        for mlo, mhi, ti in win.get((g, c), ()):
                            nc.vector.tensor_mul(
                                pr[:, 512 * j + mlo:512 * j + mhi],
                                pr[:, 512 * j + mlo:512 * j + mhi],
                                mts[ti][:, 0:mhi - mlo])
                    for j, c in enumerate(batch):
                        q0 = qlo[(g, c)]
                        nc.tensor.matmul(
                            ctx_ps[:, q0:512],
                            lhsT=VTS[c][:, h, :],
                            rhs=pr[:, 512 * j + q0:512 * j + 512],
                            start=(bi == 0 and j == 0),
                            stop=(bi == nbat - 1 and j == len(batch) - 1))
                    fill.step(1)
                    if pending_cb is not None and h == 0 and bi == 1:
                        pending_cb()
                        pending_cb = None
                # copy ctx+denom to SBUF to release the PSUM bank
                ctxs[h] = attsb.tile([65, 512], f32, tag="ctxs", bufs=8,
                                     name=f"ctxs{h}")
                nc.vector.tensor_copy(out=ctxs[h], in_=ctx_ps)
                if g == NG - 1:
                    # last group: per-head denominator finish so the tail
                    # isn't serialized behind a batched recip chain
                    denb_head(g, h, ctxs[h])
                else:
                    # gather the denom row into denoms4 at partition 32h
                    # for the batched (deferred) per-group reciprocal
                    nc.sync.dma_start(out=denoms4[32 * h:32 * h + 1, :],
                                      in_=ctxs[h][64:65, :])
            if pending_cb is not None:
                pending_cb()

            if g == NG - 1:
                return lambda: None

            def finish():
                rc4 = attsb.tile([97, 512], f32, tag="rc4", bufs=2, name="rc4")
                nc.vector.reciprocal_approx_fast(out=rc4, in_=denoms4)
                rc4b = attsb.tile([97, 512], bf16, tag="rc4b", bufs=2, name="rc4b")
                nc.vector.tensor_copy(out=rc4b, in_=rc4)
                for h in range(HPC):
                    hc, par = h // 2, h % 2
                    bc_ps = pjps.tile([128, 512], f32, tag="pj", bufs=2,
                                      name="bc_ps")
                    nc.tensor.matmul(bc_ps[0:64, :],
                                     lhsT=ones_col[32 * h:32 * h + 1, :],
                                     rhs=rc4b[32 * h:32 * h + 1, :],
                                     start=True, stop=True,
                                     tile_position=(32 * h, 0))
                    if par == 0:
                        nc.vector.tensor_mul(CTXT[hc][g][0:64, :],
                                             ctxs[h][0:64, :], bc_ps[0:64, :])
                    else:
                        st = attsb.tile([64, 512], bf16, tag="stage", bufs=2,
                                        name="st")
                        nc.vector.tensor_mul(st, ctxs[h][0:64, :], bc_ps[0:64, :])
                        nc.sync.dma_start(out=CTXT[hc][g][64:128, :], in_=st)
            return finish

        # schedule: proj(0) upfront; during attn(g) the filler queue holds
        # proj(g+1); out-projections fill the (otherwise PE-starved) late
        # groups: op(0) during attn(2), op(1)/op(2) during attn(3); proj
        # leftovers flush at the group boundary (they gate attn(g+1)).
        for _ in proj_units(0, ld0):
            pass
        pending = None
        for g in range(NG):
            if g + 1 < NG:
                ld = proj_load(g + 1)
                fill.proj.append(proj_units(g + 1, ld))
            if g == NG - 2:
                fill.op.append(outproj_units(0))
            if g == NG - 1:
                fill.op.append(outproj_units(1))
                # outproj(2) may only be emitted after denb(2) has written
                # CTXT[*][2]; chain it onto the deferred finish callback
                prev = pending

                def pending(prev=prev):
                    prev()
                    fill.op.append(outproj_units(NG - 2))
            pending = attn_group(g, pending)
            if g + 1 < NG:
                fill.flush_proj()
        pending()
        fill.drain_all()
        for _ in outproj_units(NG - 1):
            pass

    nc.compile()
    return nc


def _make_plan(mask):
    """Block plan from the actual mask (union over batches -> one SPMD plan).

    For each (q-group g, kv-chunk c) computes:
      - inclusion (any valid element),
      - qlo: first q column (within the group's 512) with any valid kv,
      - the mixed window [mlo, mhi) of q columns that need an elementwise
        mask multiply, with deduped [128, mhi-mlo<=128...] tiles.
    """
    m = np.asarray(mask[:, 0])                       # [B, LQ, LKV] bool
    chunks, qlo, wins = [], {}, {}
    uniq, order = {}, []                             # content-hash -> idx
    for g in range(NG):
        cl = [0]
        qlo[(g, 0)] = 0
        for c in range(1, NCH):
            blk = m[:, 512 * g:512 * g + 512, 128 * (c - 1):128 * c]  # [B,512,128]
            anyk = blk.any(axis=2)                   # [B, 512]
            if not anyk.any():
                continue
            cl.append(c)
            valid_cols = anyk.any(axis=0)            # union over batches
            q0 = int(np.argmax(valid_cols))
            qlo[(g, c)] = q0
            allk = blk.all(axis=2).all(axis=0)       # [512] all-valid cols
            mixed = valid_cols & ~allk
            if mixed.any():
                lo = int(np.argmax(mixed))
                hi = 512 - int(np.argmax(mixed[::-1]))
                wl = []
                # split into <=128-wide windows (mask tiles are [128,128])
                for mlo in range(lo, hi, 128):
                    mhi = min(mlo + 128, hi)
                    # per-batch tile content; dedup on the union key so all
                    # cores run the same program with per-core data
                    key = (mhi - mlo, blk[:, mlo:mhi, :].tobytes())
                    if key not in uniq:
                        uniq[key] = len(order)
                        order.append((g, c, mlo, mhi))
                    wl.append((mlo, mhi, uniq[key]))
                wins[(g, c)] = wl
        chunks.append(cl)
    return {"chunks": chunks, "qlo": qlo, "win": wins, "ntiles": len(order),
            "order": order}


def _prep_core_inputs(inputs, plan):
    """Per-core input dicts (8 cores: batch-major, then head-group)."""
    inputs_q = np.asarray(inputs["inputs_q"], dtype=np.float32)
    inputs_kv = np.asarray(inputs["inputs_kv"], dtype=np.float32)
    key_prefix = np.asarray(inputs["key_prefix"], dtype=np.float32)
    value_prefix = np.asarray(inputs["value_prefix"], dtype=np.float32)
    mask = np.asarray(inputs["mask"])
    Wq = np.asarray(inputs["Wq"], dtype=np.float32)
    Wk = np.asarray(inputs["Wk"], dtype=np.float32)
    Wv = np.asarray(inputs["Wv"], dtype=np.float32)
    Wo = np.asarray(inputs["Wo"], dtype=np.float32)

    xT = [np.ascontiguousarray(inputs_q[b].T.astype(BF16)) for b in range(B)]
    xkT = [np.ascontiguousarray(inputs_kv[b].T.astype(BF16)) for b in range(B)]

    maskblks = []
    ntiles = plan["ntiles"]
    for b in range(B):
        mb = np.zeros((max(ntiles, 1), 128, 128), np.float32)
        for i, (g, c, mlo, mhi) in enumerate(plan["order"]):
            mb[i, :, 0:mhi - mlo] = mask[
                b, 0, 512 * g + mlo:512 * g + mhi,
                128 * (c - 1):128 * c].T.astype(np.float32)
        maskblks.append(mb.astype(BF16))

    in_maps = []
    for core in range(NCORES):
        b, hg = core // HGROUPS, core % HGROUPS
        hs = slice(HPC * hg, HPC * (hg + 1))
        kpT = key_prefix[b, :, hs, :]                 # [P, 4, D]
        kpT = kpT.transpose(1, 2, 0).reshape(2, 128, P)  # [hc, (2 heads x D), P]
        kpT = np.concatenate(
            [kpT, np.zeros((2, 128, 128 - P), np.float32)], axis=2)
        im = {
            "xqT": xT[b],
            "xkvT": xkT[b],
            "wq": np.ascontiguousarray(
                (Wq[:, hs, :] / np.sqrt(D)).reshape(E, HPC * D).astype(BF16)),
            "wk": np.ascontiguousarray(Wk[:, hs, :].reshape(E, HPC * D).astype(BF16)),
            "wv": np.ascontiguousarray(Wv[:, hs, :].reshape(E, HPC * D).astype(BF16)),
            "wo": np.ascontiguousarray(Wo[hs].reshape(HPC * D, E).astype(BF16)),
            "kprefT": np.ascontiguousarray(kpT.astype(BF16)),
            "vpref": np.ascontiguousarray(np.concatenate(
                [value_prefix[b, :, hs, :],
                 np.zeros((128 - P, HPC, D), np.float32)], axis=0).astype(BF16)),
        }
        if ntiles:
            im["maskblk"] = maskblks[b]
        in_maps.append(im)
    return in_maps


def kernel(**inputs) -> np.ndarray:
    from concourse import bass_utils

    plan = _make_plan(inputs["mask"])
    key = (tuple(tuple(c) for c in plan["chunks"]),
           tuple(sorted(plan["qlo"].items())),
           tuple(sorted((k, tuple(v)) for k, v in plan["win"].items())),
           plan["ntiles"])
    if key not in _CACHE:
        _CACHE[key] = _build_module(plan)
    nc = _CACHE[key]

    in_maps = _prep_core_inputs(inputs, plan)
    res = bass_utils.run_bass_kernel_spmd(nc, in_maps, core_ids=list(range(NCORES)))

    out = np.zeros((B, LQ, E), np.float32)
    for core in range(NCORES):
        b = core // HGROUPS
        out[b] += res.results[core]["outT"].T.astype(np.float32)
    return out


# revision 26
# speedup vs baseline: 1.1585x; 1.0272x over previous
"""Multi-head dot-product attention with prefix KV, on 8 trn2 NeuronCores.

Sharding: batch (2) x head-groups (4 groups of 4 heads) = 8 cores.
Each core computes q/k/v projections for its 4 heads, flash-style
attention (scores kept transposed: [kv, L] so no on-device transposes
are needed), and a partial out-projection [E, L]; the host sums the 4
head-group partials per batch and transposes back.

v2 layout/schedule notes:
  - all matmul operands are bf16 (PSUM accumulation stays fp32): halves
    LDWEIGHTS time and input DMA, and enables the fast DVE modes.
  - kv axis padded to 2176 = 17*128: chunk 0 = [prefix(64) | dead(64)],
    chunks 1..16 = kv positions.  Dead columns are killed with a
    per-partition -1e10 bias on the chunk-0 exp.
  - exact causal q-trim: per (group, chunk) the score/exp/ctx work only
    covers the valid q-range [qlo, 512); the elementwise mask multiply
    shrinks to the (deduped) mixed window (a [128,128] triangle for the
    causal mask).
  - softmax runs without max subtraction (scores are O(1)); denominator
    comes free as a ones-column in the v weights (M=65 ctx matmul);
    per-group denominators are gathered to partitions {0,32,64,96},
    reciprocal'd with one fast-approx DVE op, and broadcast across
    partitions with K=1 outer-product matmuls.
  - q/k projections use N=512 matmuls (half the instruction count);
    projection and out-projection matmuls are interleaved between the
    score and ctx matmuls of every attention group as PE filler, so the
    tensor engine never waits on the softmax (ACT) pipeline.
"""

import numpy as np
import ml_dtypes

BF16 = ml_dtypes.bfloat16

B, LQ, LKV, E, H, D, P = 2, 2048, 2048, 1024, 16, 64, 64
NCORES = 8
HGROUPS = 4          # head groups (cores per batch)
HPC = H // HGROUPS   # heads per core = 4
KVPAD = 128 + LKV    # 2176
NCH = KVPAD // 128   # 17 chunks
NG = LQ // 512       # 4 L-groups of 512
NEG = -1.0e10

_CACHE = {}


def _build_module(plan, debug_taps=False):
    """Build the single-core Bass module (same program for all 8 cores)."""
    import concourse.bass as bass
    import concourse.tile as tile
    import concourse.mybir as mybir
    from concourse import bacc
    from contextlib import ExitStack

    f32 = mybir.dt.float32
    bf16 = mybir.dt.bfloat16
    Exp = mybir.ActivationFunctionType.Exp

    chunks = plan["chunks"]        # g -> [c...] ascending, 0 first
    qlo = plan["qlo"]              # (g,c) -> valid-q start col (0..511)
    win = plan["win"]              # (g,c) -> (mlo, mhi, tile_idx) or absent
    ntiles = plan["ntiles"]

    nc = bacc.Bacc("TRN2", target_bir_lowering=False, debug=False,
                   enable_asserts=False, num_devices=NCORES)

    xqT_d = nc.dram_tensor("xqT", [E, LQ], bf16, kind="ExternalInput").ap()
    xkvT_d = nc.dram_tensor("xkvT", [E, LKV], bf16, kind="ExternalInput").ap()
    wq_d = nc.dram_tensor("wq", [E, HPC * D], bf16, kind="ExternalInput").ap()
    wk_d = nc.dram_tensor("wk", [E, HPC * D], bf16, kind="ExternalInput").ap()
    wv_d = nc.dram_tensor("wv", [E, HPC * D], bf16, kind="ExternalInput").ap()
    wo_d = nc.dram_tensor("wo", [HPC * D, E], bf16, kind="ExternalInput").ap()
    kprefT_d = nc.dram_tensor("kprefT", [2, 128, 128], bf16, kind="ExternalInput").ap()
    vpref_d = nc.dram_tensor("vpref", [128, HPC, D], bf16, kind="ExternalInput").ap()
    if ntiles:
        maskblk_d = nc.dram_tensor("maskblk", [ntiles, 128, 128], bf16,
                                   kind="ExternalInput").ap()
    outT_d = nc.dram_tensor("outT", [E, LQ], bf16, kind="ExternalOutput").ap()

    with tile.TileContext(nc) as tc, ExitStack() as stk:
        pers = stk.enter_context(tc.tile_pool(name="pers", bufs=1))

        def ptile(shape, name, dt=bf16):
            return pers.tile(shape, dt, tag=name, name=name)

        wq_sb = ptile([128, 8, 256], "wq_sb")
        wk_sb = ptile([128, 8, 256], "wk_sb")
        wv_sb = ptile([128, 8, 256], "wv_sb")
        wo_sb = ptile([128, 2, 1024], "wo_sb")
        # per-L-group tensors (512 wide); K prefix is its own [128,128] tile
        QTS = [[ptile([128, 512], f"QT{i}g{g}") for g in range(NG)] for i in range(2)]
        KTS = [[ptile([128, 512], f"KT{i}g{g}") for g in range(NG)] for i in range(2)]
        KPR = [ptile([128, 128], f"KP{i}") for i in range(2)]
        VTS = [ptile([128, HPC, 65], f"VT{c}") for c in range(NCH)]
        CTXT = [[ptile([128, 512], f"CTXT{i}g{g}") for g in range(NG)]
                for i in range(2)]
        cb0 = ptile([128, 1], "cb0", f32)
        ones_col = ptile([128, 64], "ones_col")
        ones_f32 = ptile([128, 64], "ones_f32", f32)
        mts = [ptile([128, 128], f"mt{i}") for i in range(ntiles)]

        def kslice(hc, c):
            if c == 0:
                return KPR[hc]
            g, off = (c - 1) // 4, 128 * ((c - 1) % 4)
            return KTS[hc][g][:, off:off + 128]

        xio = stk.enter_context(tc.tile_pool(name="xio", bufs=2))

        def proj_load(g):
            l0 = 512 * g
            xq_t = xio.tile([128, 8, 512], bf16, tag="xq", bufs=2, name="xq_t")
            xkv_t = xio.tile([128, 8, 512], bf16, tag="xkv", bufs=2, name="xkv_t")
            nc.sync.dma_start(
                out=xq_t,
                in_=xqT_d.rearrange("(ec p) l -> p ec l", p=128)[:, :, l0:l0 + 512])
            nc.sync.dma_start(
                out=xkv_t,
                in_=xkvT_d.rearrange("(ec p) l -> p ec l", p=128)[:, :, l0:l0 + 512])
            return xq_t, xkv_t

        # startup DMAs ordered by first consumption: wq+x(0) gate proj(0)
        nc.sync.dma_start(out=wq_sb, in_=wq_d.rearrange("(ec p) m -> p ec m", p=128))
        ld0 = proj_load(0)
        nc.sync.dma_start(out=wk_sb, in_=wk_d.rearrange("(ec p) m -> p ec m", p=128))
        nc.sync.dma_start(out=wv_sb, in_=wv_d.rearrange("(ec p) m -> p ec m", p=128))
        nc.sync.dma_start(out=wo_sb, in_=wo_d.rearrange("(hc p) e -> p hc e", p=128))

        nc.vector.memset(cb0[0:64, :], 0.0)
        nc.vector.memset(cb0[64:128, :], NEG)
        nc.vector.memset(ones_col, 1.0)
        nc.vector.memset(ones_f32, 1.0)
        for c in range(NCH):
            nc.vector.memset(VTS[c][:, :, 64:65], 1.0)
        for i in range(ntiles):
            nc.sync.dma_start(out=mts[i], in_=maskblk_d[i])
        for hc in range(2):
            nc.sync.dma_start(out=KPR[hc], in_=kprefT_d[hc])
        nc.sync.dma_start(out=VTS[0][:, :, 0:D], in_=vpref_d)

        attps = stk.enter_context(tc.tile_pool(name="att_ps", bufs=1, space="PSUM"))
        attsb = stk.enter_context(tc.tile_pool(name="att_sb", bufs=1))
        pjps = stk.enter_context(tc.tile_pool(name="pj_ps", bufs=1, space="PSUM"))

        def proj_units(g, loaded):
            """q/k/v projection for L-group g as a sequence of 4-matmul
            quanta (yield between quanta so attention can interleave)."""
            xq_t, xkv_t = loaded
            # each unit is atomic: its 8-matmul PSUM accumulation group must
            # close before anything else can allocate from the pj ring
            for t in range(2):
                for w_sb, x_t, dst in ((wq_sb, xq_t, QTS), (wk_sb, xkv_t, KTS)):
                    ps = pjps.tile([128, 512], f32, tag="pj", bufs=2, name="ps_p")
                    for ec in range(8):
                        nc.tensor.matmul(
                            ps, lhsT=w_sb[:, ec, 128 * t:128 * t + 128],
                            rhs=x_t[:, ec, :], start=(ec == 0), stop=(ec == 7))
                    nc.vector.tensor_copy(out=dst[t][g], in_=ps)
                    yield
            for sub in range(4):
                ps = pjps.tile([128, 512], f32, tag="pj", bufs=2, name="ps_v")
                for ec in range(8):
                    nc.tensor.matmul(
                        ps[:, 0:256], lhsT=xkv_t[:, ec, 128 * sub:128 * sub + 128],
                        rhs=wv_sb[:, ec, :], start=(ec == 0), stop=(ec == 7))
                nc.vector.tensor_copy(
                    out=VTS[4 * g + sub + 1][:, :, 0:D],
                    in_=ps[:, 0:256].rearrange("p (h d) -> p h d", h=HPC))
                yield

        op_n = [0]

        def outproj_units(g):
            """out-projection for L-group g: 8 units of (2 matmuls + copy)."""
            gl = 512 * g
            for et in range(8):
                ops = pjps.tile([128, 512], f32, tag="pj", bufs=2, name="ops")
                for hc in range(2):
                    nc.tensor.matmul(
                        ops, lhsT=wo_sb[:, hc, 128 * et:128 * et + 128],
                        rhs=CTXT[hc][g], start=(hc == 0), stop=(hc == 1))
                ot = attsb.tile([128, 512], bf16, tag="ostage", bufs=6, name="ot")
                # PSUM->SBUF copy on DVE (ACT is saturated by the softmax
                # exps mid-run); the last group's units alternate onto ACT,
                # which is idle during the tail
                if g == NG - 1 and op_n[0] % 2 == 0:
                    nc.scalar.copy(ot, ops)
                else:
                    nc.vector.tensor_copy(out=ot, in_=ops)
                op_n[0] += 1
                nc.sync.dma_start(
                    out=outT_d[128 * et:128 * et + 128, gl:gl + 512], in_=ot)
                yield

        class FillerQ:
            """Two queues of generators (proj must flush at group
            boundaries, outproj carries over); step() advances one quantum,
            preferring proj."""
            def __init__(self):
                self.proj = []
                self.op = []

            @staticmethod
            def _step_one(q):
                while q:
                    try:
                        next(q[0])
                        return True
                    except StopIteration:
                        q.pop(0)
                return False

            def step(self, n=1):
                for _ in range(n):
                    if not self._step_one(self.proj):
                        self._step_one(self.op)

            def flush_proj(self):
                while self._step_one(self.proj):
                    pass

            def drain_all(self):
                self.flush_proj()
                while self._step_one(self.op):
                    pass

        fill = FillerQ()

        def denb_head(g, h, cx):
            """Per-head denominator finish: partition-broadcast the raw
            denom row via a K=1 matmul, fast reciprocal (at partition
            offset 0 — the custom DVE op needs that), CTXT scaling."""
            hc, par = h // 2, h % 2
            bc_ps = pjps.tile([128, 512], f32, tag="pj", bufs=2, name="bc_ps")
            nc.tensor.matmul(bc_ps[0:64, :], lhsT=ones_f32[64:65, :],
                             rhs=cx[64:65, :], start=True, stop=True,
                             tile_position=(64, 0))
            rcs = attsb.tile([64, 512], f32, tag="rcs", bufs=2, name="rcs")
            nc.vector.reciprocal_approx_fast(out=rcs, in_=bc_ps[0:64, :])
            if par == 0:
                nc.vector.tensor_mul(CTXT[hc][g][0:64, :], cx[0:64, :], rcs)
            else:
                st = attsb.tile([64, 512], bf16, tag="stage", bufs=2, name="st")
                nc.vector.tensor_mul(st, cx[0:64, :], rcs)
                nc.sync.dma_start(out=CTXT[hc][g][64:128, :], in_=st)

        def attn_group(g, pending_cb=None):
            """Score/softmax/ctx for group g.  Returns a closure that
            finishes the softmax denominators (reciprocal + broadcast +
            CTXT scaling); the caller runs it early in the NEXT group so
            the in-order PE queue never stalls on the recip sem chain."""
            cs = chunks[g]
            batches = [[cs[0]]] + [cs[1 + i:3 + i] for i in range(0, len(cs) - 1, 2)]
            ctxs = {}
            if g < NG - 1:
                denoms4 = attsb.tile([97, 512], f32, tag="den4", bufs=2,
                                     name="denoms4")
                nc.vector.memset(denoms4, 1.0)
            nbat = len(batches)
            for h in range(HPC):
                hc, prow = h // 2, 64 * (h % 2)
                ctx_ps = attps.tile([65, 512], f32, tag="ctx", bufs=2,
                                    name=f"ctx{h}")
                for bi, batch in enumerate(batches):
                    sc = attps.tile([128, 1024], f32, tag="sc", bufs=2,
                                    name=f"sc{h}")
                    for j, c in enumerate(batch):
                        q0 = qlo[(g, c)]
                        nc.tensor.matmul(
                            sc[:, 512 * j + q0:512 * j + 512],
                            lhsT=kslice(hc, c)[prow:prow + 64, :],
                            rhs=QTS[hc][g][prow:prow + 64, q0:512],
                            start=True, stop=True)
                    pr = attsb.tile([128, 1024], bf16, tag="pr", bufs=4,
                                    name=f"pr{h}")
                    e0 = qlo[(g, batch[0])]
                    we = 512 * len(batch)
                    if batch[0] == 0:
                        nc.scalar.activation(pr[:, e0:we], sc[:, e0:we],
                                             Exp, bias=cb0[:, 0:1])
                    else:
                        nc.scalar.activation(pr[:, e0:we], sc[:, e0:we], Exp)
                    fill.step(1)
                    for j, c in enumerate(batch):
                        for mlo, mhi, ti in win.get((g, c), ()):
                            nc.vector.tensor_mul(
                                pr[:, 512 * j + mlo:512 * j + mhi],
                                pr[:, 512 * j + mlo:512 * j + mhi],
                                mts[ti][:, 0:mhi - mlo])
                    for j, c in enumerate(batch):
                        q0 = qlo[(g, c)]
                        nc.tensor.matmul(
                            ctx_ps[:, q0:512],
                            lhsT=VTS[c][:, h, :],
                            rhs=pr[:, 512 * j + q0:512 * j + 512],
                            start=(bi == 0 and j == 0),
                            stop=(bi == nbat - 1 and j == len(batch) - 1))
                    fill.step(1)
                    if pending_cb is not None and h == 0 and bi == 1:
                        pending_cb()
                        pending_cb = None
                # copy ctx+denom to SBUF to release the PSUM bank
                ctxs[h] = attsb.tile([65, 512], f32, tag="ctxs", bufs=8,
                                     name=f"ctxs{h}")
                nc.vector.tensor_copy(out=ctxs[h], in_=ctx_ps)
                if g == NG - 1:
                    # last group: per-head denominator finish so the tail
                    # isn't serialized behind a batched recip chain
                    denb_head(g, h, ctxs[h])
                else:
                    # gather the denom row into denoms4 at partition 32h
                    # for the batched (deferred) per-group reciprocal
                    nc.sync.dma_start(out=denoms4[32 * h:32 * h + 1, :],
                                      in_=ctxs[h][64:65, :])
            if pending_cb is not None:
                pending_cb()

            if g == NG - 1:
                return lambda: None

            def finish():
                rc4 = attsb.tile([97, 512], f32, tag="rc4", bufs=2, name="rc4")
                nc.vector.reciprocal_approx_fast(out=rc4, in_=denoms4)
                rc4b = attsb.tile([97, 512], bf16, tag="rc4b", bufs=2, name="rc4b")
                nc.vector.tensor_copy(out=rc4b, in_=rc4)
                for h in range(HPC):
                    hc, par = h // 2, h % 2
                    bc_ps = pjps.tile([128, 512], f32, tag="pj", bufs=2,
                                      name="bc_ps")
                    nc.tensor.matmul(bc_ps[0:64, :],
                                     lhsT=ones_col[32 * h:32 * h + 1, :],
                                     rhs=rc4b[32 * h:32 * h + 1, :],
                                     start=True, stop=True,
                                     tile_position=(32 * h, 0))
                    if par == 0:
                        nc.vector.tensor_mul(CTXT[hc][g][0:64, :],
                                             ctxs[h][0:64, :], bc_ps[0:64, :])
                    else:
                        st = attsb.tile([64, 512], bf16, tag="stage", bufs=2,
                                        name="st")
                        nc.vector.tensor_mul(st, ctxs[h][0:64, :], bc_ps[0:64, :])
                        nc.sync.dma_start(out=CTXT[hc][g][64:128, :], in_=st)
            return finish

        # schedule: proj(0) upfront; during attn(g) the filler queue holds
        # proj(g+1); out-projections fill the (otherwise PE-starved) late
        # groups: op(0) during attn(2), op(1)/op(2) during attn(3); proj
        # leftovers flush at the group boundary (they gate attn(g+1)).
        for _ in proj_units(0, ld0):
            pass
        pending = None
        for g in range(NG):
            if g + 1 < NG:
                ld = proj_load(g + 1)
                fill.proj.append(proj_units(g + 1, ld))
            if g == NG - 2:
                fill.op.append(outproj_units(0))
            if g == NG - 1:
                fill.op.append(outproj_units(1))
                # outproj(2) may only be emitted after denb(2) has written
                # CTXT[*][2]; chain it onto the deferred finish callback
                prev = pending

                def pending(prev=prev):
                    prev()
                    fill.op.append(outproj_units(NG - 2))
            pending = attn_group(g, pending)
            if g + 1 < NG:
                fill.flush_proj()
        pending()
        fill.drain_all()
        for _ in outproj_units(NG - 1):
            pass

    nc.compile()
    return nc


def _make_plan(mask):
    """Block plan from the actual mask (union over batches -> one SPMD plan).

    For each (q-group g, kv-chunk c) computes:
      - inclusion (any valid element),
      - qlo: first q column (within the group's 512) with any valid kv,
      - the mixed window [mlo, mhi) of q columns that need an elementwise
        mask multiply, with deduped [128, mhi-mlo<=128...] tiles.
    """
    m = np.asarray(mask[:, 0])                       # [B, LQ, LKV] bool
    chunks, qlo, wins = [], {}, {}
    uniq, order = {}, []                             # content-hash -> idx
    for g in range(NG):
        cl = [0]
        qlo[(g, 0)] = 0
        for c in range(1, NCH):
            blk = m[:, 512 * g:512 * g + 512, 128 * (c - 1):128 * c]  # [B,512,128]
            anyk = blk.any(axis=2)                   # [B, 512]
            if not anyk.any():
                continue
            cl.append(c)
            valid_cols = anyk.any(axis=0)            # union over batches
            q0 = int(np.argmax(valid_cols))
            qlo[(g, c)] = q0
            allk = blk.all(axis=2).all(axis=0)       # [512] all-valid cols
            mixed = valid_cols & ~allk
            if mixed.any():
                lo = int(np.argmax(mixed))
                hi = 512 - int(np.argmax(mixed[::-1]))
                wl = []
                # split into <=128-wide windows (mask tiles are [128,128])
                for mlo in range(lo, hi, 128):
                    mhi = min(mlo + 128, hi)
                    # per-batch tile content; dedup on the union key so all
                    # cores run the same program with per-core data
                    key = (mhi - mlo, blk[:, mlo:mhi, :].tobytes())
                    if key not in uniq:
                        uniq[key] = len(order)
                        order.append((g, c, mlo, mhi))
                    wl.append((mlo, mhi, uniq[key]))
                wins[(g, c)] = wl
        chunks.append(cl)
    return {"chunks": chunks, "qlo": qlo, "win": wins, "ntiles": len(order),
            "order": order}


def _prep_core_inputs(inputs, plan):
    """Per-core input dicts (8 cores: batch-major, then head-group)."""
    inputs_q = np.asarray(inputs["inputs_q"], dtype=np.float32)
    inputs_kv = np.asarray(inputs["inputs_kv"], dtype=np.float32)
    key_prefix = np.asarray(inputs["key_prefix"], dtype=np.float32)
    value_prefix = np.asarray(inputs["value_prefix"], dtype=np.float32)
    mask = np.asarray(inputs["mask"])
    Wq = np.asarray(inputs["Wq"], dtype=np.float32)
    Wk = np.asarray(inputs["Wk"], dtype=np.float32)
    Wv = np.asarray(inputs["Wv"], dtype=np.float32)
    Wo = np.asarray(inputs["Wo"], dtype=np.float32)

    xT = [np.ascontiguousarray(inputs_q[b].T.astype(BF16)) for b in range(B)]
    xkT = [np.ascontiguousarray(inputs_kv[b].T.astype(BF16)) for b in range(B)]

    maskblks = []
    ntiles = plan["ntiles"]
    for b in range(B):
        mb = np.zeros((max(ntiles, 1), 128, 128), np.float32)
        for i, (g, c, mlo, mhi) in enumerate(plan["order"]):
            mb[i, :, 0:mhi - mlo] = mask[
                b, 0, 512 * g + mlo:512 * g + mhi,
                128 * (c - 1):128 * c].T.astype(np.float32)
        maskblks.append(mb.astype(BF16))

    in_maps = []
    for core in range(NCORES):
        b, hg = core // HGROUPS, core % HGROUPS
        hs = slice(HPC * hg, HPC * (hg + 1))
        kpT = key_prefix[b, :, hs, :]                 # [P, 4, D]
        kpT = kpT.transpose(1, 2, 0).reshape(2, 128, P)  # [hc, (2 heads x D), P]
        kpT = np.concatenate(
            [kpT, np.zeros((2, 128, 128 - P), np.float32)], axis=2)
        im = {
            "xqT": xT[b],
            "xkvT": xkT[b],
            "wq": np.ascontiguousarray(
                (Wq[:, hs, :] / np.sqrt(D)).reshape(E, HPC * D).astype(BF16)),
            "wk": np.ascontiguousarray(Wk[:, hs, :].reshape(E, HPC * D).astype(BF16)),
            "wv": np.ascontiguousarray(Wv[:, hs, :].reshape(E, HPC * D).astype(BF16)),
            "wo": np.ascontiguousarray(Wo[hs].reshape(HPC * D, E).astype(BF16)),
            "kprefT": np.ascontiguousarray(kpT.astype(BF16)),
            "vpref": np.ascontiguousarray(np.concatenate(
                [value_prefix[b, :, hs, :],
                 np.zeros((128 - P, HPC, D), np.float32)], axis=0).astype(BF16)),
        }
        if ntiles:
            im["maskblk"] = maskblks[b]
        in_maps.append(im)
    return in_maps


def kernel(**inputs) -> np.ndarray:
    from concourse import bass_utils

    plan = _make_plan(inputs["mask"])
    key = (tuple(tuple(c) for c in plan["chunks"]),
           tuple(sorted(plan["qlo"].items())),
           tuple(sorted((k, tuple(v)) for k, v in plan["win"].items())),
           plan["ntiles"])
    if key not in _CACHE:
        _CACHE[key] = _build_module(plan)
    nc = _CACHE[key]

    in_maps = _prep_core_inputs(inputs, plan)
    res = bass_utils.run_bass_kernel_spmd(nc, in_maps, core_ids=list(range(NCORES)))

    out = np.zeros((B, LQ, E), np.float32)
    for core in range(NCORES):
        b = core // HGROUPS
        out[b] += res.results[core]["outT"].T.astype(np.float32)
    return out
